# revision 11
# baseline (speedup 1.0000x reference)
"""BioSelfAttention on 8 TRN2 NeuronCores.

Full inputs Q,K,V: (B=2, H=8, T=256, D=64) f32. Data-parallel over the 16
(b,h) pairs: core c owns flat pairs {2c, 2c+1}.

The module constant-folds: its output is the constant 1/16384 for every
finite input, so the device kernel is a memset + store.

Why (exact, not approximate):
  * The WTA update is r <- softmax((r + r @ W.T)/temp) with W = inh*ones
    + (exc-inh)*I, i.e. softmax(3r - 0.9*sum(r)); the -0.9*sum(r) term is
    constant per row and cancels inside softmax, leaving r <- softmax(3r).
    Over N elements this map contracts deviations from uniform by ~3/N per
    step (Jacobian 3(diag(p) - p p^T), spectral radius ~3/N near uniform,
    and globally every state collapses toward uniform since
    exp(3)/(exp(3)+N-1) << 1 for N >= 256).  With 20 reference iterations
    the state reaches the uniform vector BITWISE: once fp32 rounding makes
    all entries tie, softmax gives exp(0) = 1 per element and the sum
    (a power of two: 256 or 16384) is exact, so uniform is an exact fixed
    point of the fp32 computation.
  * WTA1 (N=256) therefore yields rates_inh == 1/256 exactly.  Then
    J_v = V/256 with |V| ~ randn, far below the LIF threshold (a spike
    needs J >= 1/(1-0.95^100) ~ 1.006), so context == 0 identically.
  * WTA2 (N=16384) of the all-zero vector: softmax(0) = 1/16384 exactly
    (sum of 16384 ones is exact), a fixed point of every later iteration.
  * Even for huge inputs where J_v does spike, WTA2's 20 iterations
    contract by 3/16384 per step -> bitwise uniform 1/16384 regardless.
    The output is input-independent for all finite inputs.

The previous full on-device implementation of the LIF/WTA pipeline (kept
in git history / kernel_baseline.py during development) reproduces this
constant bitwise in 53 us; this kernel emits it directly.

Per core: memset a [128,256] SBUF tile to 1/16384 and DMA it to the
core's (2,256,64) output slice; with out viewed as
(h, p*4+q, d) -> partition (h p), free (q d), each of the 128 partitions
writes one contiguous 1 KiB DRAM line.
"""

import numpy as np
import concourse.bacc as bacc
import concourse.mybir as mybir
import concourse.tile as tile
from concourse.bass_utils import run_bass_kernel_spmd

F32 = mybir.dt.float32
B, H, T, D = 2, 8, 256, 64
N_CORES = 8
UNIFORM = 1.0 / 16384.0


import os

VARIANT = os.environ.get("KVARIANT", "tile2")


def _build_body_tile2(nc, tc, out):
    """out: DRAM AP of per-core shape (2,256,64) f32."""
    with tc.tile_pool(name="pool", bufs=1) as pool:
        outt = pool.tile([128, 256], F32)
        # Split the memset and the store across two engine queues so the
        # two 64 KiB halves of the 128 KiB output stream in parallel.
        nc.vector.memset(outt[:, 0:128], UNIFORM)
        nc.gpsimd.memset(outt[:, 128:256], UNIFORM)
        # out[h, p*4+q, d] <-> partition (h p), free (q d): contiguous
        # 1 KiB per partition line.
        dst = out.rearrange("h (p q) d -> (h p) (q d)", q=4)
        nc.sync.dma_start(out=dst[0:128, 0:128], in_=outt[:, 0:128])
        nc.scalar.dma_start(out=dst[0:128, 128:256], in_=outt[:, 128:256])


_NC_CACHE = {}


def _build_nc():
    if "nc" in _NC_CACHE:
        return _NC_CACHE["nc"]
    nc = bacc.Bacc(None, target_bir_lowering=False, debug=False)
    out = nc.dram_tensor("out", [2, T, D], F32, kind="ExternalOutput")
    if VARIANT == "tile2":
        with tile.TileContext(nc) as tc:
            _build_body_tile2(nc, tc, out.ap())
    elif VARIANT == "dram_tile":
        # Single contiguous 128 KiB DRAM->DRAM DMA from a NEFF-embedded
        # constant; TileContext supplies the DGE sync info.
        csrc = nc.inline_tensor(
            np.full((2, T, D), UNIFORM, dtype=np.float32), name="csrc")
        with tile.TileContext(nc) as tc:
            nc.sync.dma_start(out=out.ap()[:, :, :], in_=csrc.ap()[:, :, :])
    elif VARIANT == "raw1":
        # Rawest form: one DRAM->DRAM DMA on SP from the NEFF-embedded
        # constant. The .then_inc supplies the DGE sync info codegen
        # requires; nothing waits on it — completion is guaranteed by the
        # framework's end-of-kernel InstDrain on SP before the final
        # barrier/notify.
        csrc = nc.inline_tensor(
            np.full((2, T, D), UNIFORM, dtype=np.float32), name="csrc")
        sem = nc.alloc_semaphore(name="dma_done")
        nc.sync.dma_start(
            out=out.ap()[:, :, :], in_=csrc.ap()[:, :, :]).then_inc(sem, 16)
    elif VARIANT == "raw2":
        # Same, split across the two hw-DGE engines.
        csrc = nc.inline_tensor(
            np.full((2, T, D), UNIFORM, dtype=np.float32), name="csrc")
        s0 = nc.alloc_semaphore(name="dma_done0")
        s1 = nc.alloc_semaphore(name="dma_done1")
        nc.sync.dma_start(
            out=out.ap()[0], in_=csrc.ap()[0]).then_inc(s0, 16)
        nc.scalar.dma_start(
            out=out.ap()[1], in_=csrc.ap()[1]).then_inc(s1, 16)
    elif VARIANT == "raw2sp":
        # raw2 with single_packet: one ring posting per DMA instead of a
        # 16-way split; descriptor generation and ring drain get cheaper,
        # the transfer itself serializes on one ring per engine.
        csrc = nc.inline_tensor(
            np.full((2, T, D), UNIFORM, dtype=np.float32), name="csrc")
        s0 = nc.alloc_semaphore(name="dma_done0")
        s1 = nc.alloc_semaphore(name="dma_done1")
        nc.sync.dma_start(
            out=out.ap()[0], in_=csrc.ap()[0],
            single_packet=True).then_inc(s0, 16)
        nc.scalar.dma_start(
            out=out.ap()[1], in_=csrc.ap()[1],
            single_packet=True).then_inc(s1, 16)
    elif VARIANT == "raw2nb":
        # raw2 plus: prune the post-preamble all-engine gather/release
        # barrier from our own emitted main block. The DMAs depend only on
        # the NEFF-load-time constant and the cleared-by-preamble output
        # buffer, so the barrier only adds cross-engine semaphore latency.
        csrc = nc.inline_tensor(
            np.full((2, T, D), UNIFORM, dtype=np.float32), name="csrc")
        s0 = nc.alloc_semaphore(name="dma_done0")
        s1 = nc.alloc_semaphore(name="dma_done1")
        eng0 = nc.scalar if os.environ.get("KNB_SC", "0") == "1" else nc.sync
        eng0.dma_start(
            out=out.ap()[0], in_=csrc.ap()[0]).then_inc(s0, 16)
        nc.scalar.dma_start(
            out=out.ap()[1], in_=csrc.ap()[1]).then_inc(s1, 16)
        nc.compile()
        blk = nc.main_func.blocks[0]
        def _is_barrier(i):
            si = getattr(i, 'sync_info', None)
            if si is not None:
                for x in list(si.on_wait) + list(si.on_update):
                    if 'barrier_' in (x.ant_name or ''):
                        return True
            return False
        keep, dropped = [], 0
        insts = list(blk.instructions)
        for idx, i in enumerate(insts):
            bare_pool_drain = (
                type(i).__name__ == 'InstDrain'
                and str(getattr(i, 'engine', '')) == 'EngineType.Pool'
                and idx + 1 < len(insts) and _is_barrier(insts[idx + 1]))
            if _is_barrier(i) or bare_pool_drain:
                dropped += 1
                continue
            keep.append(i)
        assert dropped == 11, f"expected 11 barrier insts, dropped {dropped}"
        blk.instructions[:] = keep
        _NC_CACHE["nc"] = nc
        return nc
    elif VARIANT == "gpsimd_raw2":
        # Software-DGE: gpsimd posts the descriptors itself (no
        # PSEUDO_DMA_DIRECT2D ucode), DRAM->DRAM from the NEFF constant.
        csrc = nc.inline_tensor(
            np.full((2, T, D), UNIFORM, dtype=np.float32), name="csrc")
        sem = nc.alloc_semaphore(name="dma_done")
        nc.gpsimd.dma_start(
            out=out.ap()[:, :, :], in_=csrc.ap()[:, :, :]).then_inc(sem, 16)
    elif VARIANT == "gpsimd_raw":
        # Single engine, raw bass: memset then DMA in program order on
        # gpsimd; no cross-engine sync needed.
        with nc.sbuf_tensor("outt", [128, 256], F32) as outt:
            dst = out.ap().rearrange("h (p q) d -> (h p) (q d)", q=4)
            nc.gpsimd.memset(outt.ap()[:, :], UNIFORM)
            nc.gpsimd.dma_start(out=dst[:, :], in_=outt.ap()[:, :])
    elif VARIANT == "dram_raw":
        # No SBUF, no memset, no cross-engine sync: the constant output
        # block is embedded in the NEFF (loaded to HBM at model-load time)
        # and the kernel is a single contiguous 128 KiB DRAM->DRAM DMA.
        csrc = nc.inline_tensor(
            np.full((2, T, D), UNIFORM, dtype=np.float32), name="csrc")
        nc.sync.dma_start(out=out.ap()[:, :, :], in_=csrc.ap()[:, :, :])
    elif VARIANT == "sync_raw":
        # memset on gpsimd, DMA on sync hw-DGE with a manual semaphore.
        with nc.sbuf_tensor("outt", [128, 256], F32) as outt:
            dst = out.ap().rearrange("h (p q) d -> (h p) (q d)", q=4)
            sem = nc.semaphore("ms_done")
            nc.gpsimd.memset(outt.ap()[:, :], UNIFORM)
            nc.gpsimd.sem_inc(sem, 1)
            nc.sync.wait_ge(sem, 1)
            nc.sync.dma_start(out=dst[:, :], in_=outt.ap()[:, :])
    else:
        raise ValueError(VARIANT)
    nc.compile()
    _NC_CACHE["nc"] = nc
    return nc


def _run(Q, K, V, trace=False, **trace_kwargs):
    nc = _build_nc()
    in_maps = [{} for _ in range(N_CORES)]
    res = run_bass_kernel_spmd(nc, in_maps, list(range(N_CORES)),
                               trace=trace, **trace_kwargs)
    out = np.concatenate([res.results[c]["out"] for c in range(N_CORES)],
                         axis=0)
    return out.reshape(B, H, T, D), res


def kernel(Q, K, V):
    out, _ = _run(Q, K, V)
    return out
